# revision 36
# baseline (speedup 1.0000x reference)
"""Extended S5 SSM on 8 Trainium2 NeuronCores (Bass/Tile).

Algorithm (sequence-parallel, L sharded across 8 cores, feature-on-partition
layout everywhere so no on-device transposes are needed):

  pass 1:  Bu = B @ u^T               (PE, bf16)
           complex diagonal scan over L via rotation factorization:
             x_k = lam*x_{k-1} + b_k,  lam = m*e^{i th}
             y_k = e^{-ik th} x_k  =>  y_k = m*y_{k-1} + e^{-ik th} b_k
           i.e. two independent REAL first-order scans (tensor_tensor_scan)
           per P-lane after a complex "rotation" of the input.  Subchunks of
           T steps chain through tiny per-partition init rotations; cores
           chain through one 8 KB AllGather of end-states (the carry), with
           the homogeneous correction  y += m^k * G  applied afterwards.
  low rank: q = F x_prev, p = q Delta, Ep = E p  (PE)
  pass 2:  scan of (Bu + Ep-rotated), same machinery, second AllGather
  out:     out^T = Cre xre2 - Cim xim2 + D*u^T   (PE + fused stt)

v2: all host inputs packed so each parameter family loads in ONE DMA
(HWDGE issue time dominated the baseline); bf16 weights/activations for
the big matmuls; paired [re|im] spill tiles; packed output stores;
phase-2/3 parameters prefetched during earlier phases.
"""

import sys

import numpy as np

for _p in ("/opt/trn_rl_repo", "/root/.axon_site/_ro/trn_rl_repo"):
    if _p not in sys.path:
        sys.path.append(_p)

try:
    import ml_dtypes
except ImportError:
    ml_dtypes = None

# ---- problem geometry (hardcoded; harness contract) ----
L, H, P, R = 16384, 1024, 1024, 512
NCORES = 8

CFG_FULL = dict(L=16384, T=512, GR=512)

_PROG_CACHE = {}

# smalls blob per-pt column layout (f32)
SM_W = 30
SM_MVEC, SM_COST, SM_SINT, SM_COST1, SM_SINT1, SM_ZCOL = 0, 1, 2, 3, 4, 5
SM_LPR, SM_LPI = 6, 10          # 4 wide each (NSUB)
SM_CRE, SM_CIM = 14, 22         # 8 wide each (NCORES)


# ======================================================================
# device program
# ======================================================================

def _emit(nc, tc, io, cfg):
    import concourse.mybir as mybir

    f32 = mybir.dt.float32
    f32r = mybir.dt.float32r
    bf16 = mybir.dt.bfloat16
    OP = mybir.AluOpType

    T = cfg["T"]
    GR = cfg["GR"]
    S = cfg["L"] // NCORES
    NSUB = S // T
    NG = S // GR
    KH = H // 128
    PTP = P // 128
    KR = R // 128
    HT = H // 128

    V = nc.vector
    G = nc.gpsimd
    A = nc.scalar

    from contextlib import ExitStack

    _ph1es = ExitStack()
    with (
        tc.tile_pool(name="persist", bufs=1) as persist,
        tc.tile_pool(name="glue", bufs=1) as glue,
        tc.tile_pool(name="dram", bufs=1, space="DRAM") as dram,
    ):
        # x1 results live until end of phase 2; opened first for LIFO pool order
        _x1es = ExitStack()
        x1r = _x1es.enter_context(tc.tile_pool(name="x1r", bufs=1))

        # phase-1-lifetime pool (B weights + u tiles; freed before C prefetch)
        ph1w = _ph1es.enter_context(tc.tile_pool(name="ph1w", bufs=1))

        # ---------- critical-path first loads: u group 0, then B chunks ----------
        ut0 = ph1w.tile([128, KH * GR], bf16, name="ut0", tag="ut0")
        nc.sync.dma_start(
            ut0[:].rearrange("p (k t) -> p k t", t=GR),
            io["uT"].ap()[:, :, 0:GR],
        )
        bt = ph1w.tile([128, 2 * KH * P], bf16, name="bt", tag="bt")
        for ch in range(4):
            w4 = KH * P // 2
            nc.sync.dma_start(
                bt[:, ch * w4 : (ch + 1) * w4], io["BT"].ap()[:, ch * w4 : (ch + 1) * w4]
            )

        # ---------- tables (single DMA each) ----------
        ctab = persist.tile([128, PTP * T], bf16, name="ctab", tag="ctab")
        nc.sync.dma_start(ctab[:], io["ctab"].ap())
        stab = persist.tile([128, PTP * T], bf16, name="stab", tag="stab")
        nc.sync.dma_start(stab[:], io["stab"].ap())
        smalls = persist.tile([128, PTP * SM_W], f32, name="smalls", tag="smalls")
        nc.sync.dma_start(smalls[:], io["smalls"].ap())

        def sm(pt, c0, w=1):
            return smalls[:, pt * SM_W + c0 : pt * SM_W + c0 + w]

        def ctab_s(pt):
            return ctab[:, pt * T : (pt + 1) * T]

        def stab_s(pt):
            return stab[:, pt * T : (pt + 1) * T]

        # remaining u groups (phase 1 streams them; emitted early, deps none)
        ut_rest = []
        for gi in range(1, NG):
            t = ph1w.tile([128, KH * GR], bf16, name=f"ut{gi}", tag=f"ut{gi}")
            nc.sync.dma_start(
                t[:].rearrange("p (k t) -> p k t", t=GR),
                io["uT"].ap()[:, :, gi * GR : (gi + 1) * GR],
            )
            ut_rest.append(t)

        # ---------- phase-2/3 parameter prefetch (issued early; consumed later) ----------
        mctab = persist.tile([128, PTP * T], bf16, name="mctab", tag="mctab")
        nc.sync.dma_start(mctab[:], io["mctab"].ap())
        mstab = persist.tile([128, PTP * T], bf16, name="mstab", tag="mstab")
        nc.sync.dma_start(mstab[:], io["mstab"].ap())
        mptab = persist.tile([128, PTP * T], bf16, name="mptab", tag="mptab")
        nc.sync.dma_start(mptab[:], io["mptab"].ap())
        dvec = persist.tile([128, HT], f32, name="dvec", tag="dvec")
        nc.sync.dma_start(dvec[:], io["Dvec"].ap())

        # ---------- DRAM scratch ----------
        w_sp = [dram.tile([128, 2 * S], bf16, name=f"w_sp{pt}", tag=f"w_sp{pt}") for pt in range(PTP)]
        y2_sp = [dram.tile([128, 2 * S], bf16, name=f"y2_sp{pt}", tag=f"y2_sp{pt}") for pt in range(PTP)]
        xe_in = [dram.tile([128, 2 * PTP], f32, name=f"xe_in{e}", tag=f"xe_in{e}") for e in range(2)]
        xe_out = [
            dram.tile([NCORES * 128, 2 * PTP], f32, addr_space="Shared", name=f"xe_out{e}", tag=f"xe_out{e}")
            for e in range(2)
        ]

        # ---------- persistent small state ----------
        i1re = [glue.tile([128, 1], f32, name=f"i1re{pt}", tag=f"i1re{pt}") for pt in range(PTP)]
        i1im = [glue.tile([128, 1], f32, name=f"i1im{pt}", tag=f"i1im{pt}") for pt in range(PTP)]
        i2re = [glue.tile([128, 1], f32, name=f"i2re{pt}", tag=f"i2re{pt}") for pt in range(PTP)]
        i2im = [glue.tile([128, 1], f32, name=f"i2im{pt}", tag=f"i2im{pt}") for pt in range(PTP)]
        nxb = [glue.tile([128, 1], f32, name=f"nxb{pt}", tag=f"nxb{pt}") for pt in range(PTP)]
        es_pk = [glue.tile([128, 2 * PTP], f32, name=f"es_pk{e}", tag=f"es_pk{e}") for e in range(2)]
        greA_t = [glue.tile([128, PTP, 1], f32, name=f"g{e}reA", tag=f"g{e}reA") for e in range(2)]
        gimA_t = [glue.tile([128, PTP, 1], f32, name=f"g{e}imA", tag=f"g{e}imA") for e in range(2)]
        GreA_t = [glue.tile([128, PTP, NSUB], f32, name=f"G{e}reA", tag=f"G{e}reA") for e in range(2)]
        GimA_t = [glue.tile([128, PTP, NSUB], f32, name=f"G{e}imA", tag=f"G{e}imA") for e in range(2)]
        nGimA_t = glue.tile([128, PTP, NSUB], f32, name="nGimA", tag="nGimA")

        def subchunk_glue(eng, y_re_ap, y_im_ap, ire, iim, pt, sc_pool):
            # init_{tau+1} = e^{i T th} * y_last  (per-partition rotation)
            yr = y_re_ap[:, T - 1 : T]
            yi = y_im_ap[:, T - 1 : T]
            a = sc_pool.tile([128, 1], f32, tag="ga")
            b = sc_pool.tile([128, 1], f32, tag="gb")
            eng.tensor_scalar_mul(a[:], yr, sm(pt, SM_COST))
            eng.tensor_scalar_mul(b[:], yi, sm(pt, SM_SINT))
            c = sc_pool.tile([128, 1], f32, tag="gc")
            d = sc_pool.tile([128, 1], f32, tag="gd")
            eng.tensor_scalar_mul(c[:], yr, sm(pt, SM_SINT))
            eng.tensor_scalar_mul(d[:], yi, sm(pt, SM_COST))
            eng.tensor_tensor(ire[:], a[:], b[:], op=OP.subtract)
            eng.tensor_tensor(iim[:], c[:], d[:], op=OP.add)

        def end_state(eng, y_re_ap, y_im_ap, pt, sc_pool, exi):
            # x_end = e^{i (T-1) th} * y_last -> packed (re,im) per pt
            yr = y_re_ap[:, T - 1 : T]
            yi = y_im_ap[:, T - 1 : T]
            a = sc_pool.tile([128, 1], f32, tag="ea")
            b = sc_pool.tile([128, 1], f32, tag="eb")
            eng.tensor_scalar_mul(a[:], yr, sm(pt, SM_COST1))
            eng.tensor_scalar_mul(b[:], yi, sm(pt, SM_SINT1))
            pk = es_pk[exi]
            eng.tensor_tensor(pk[:, 2 * pt : 2 * pt + 1], a[:], b[:], op=OP.subtract)
            eng.tensor_scalar_mul(a[:], yr, sm(pt, SM_SINT1))
            eng.tensor_scalar_mul(b[:], yi, sm(pt, SM_COST1))
            eng.tensor_tensor(pk[:, 2 * pt + 1 : 2 * pt + 2], a[:], b[:], op=OP.add)
            if pt == PTP - 1:
                nc.sync.dma_start(xe_in[exi][:, :], pk[:])

        # ==============================================================
        # PHASE 1: Bu matmuls, rotation, local scan, spills, end states
        # ==============================================================
        x1res = {}
        with (
            tc.tile_pool(name="p1", bufs=2) as p1,
            tc.tile_pool(name="ps1", bufs=4, space="PSUM") as ps1,
        ):
            for pt in range(PTP):
                V.memset(i1re[pt][:], 0.0)
                V.memset(i1im[pt][:], 0.0)

            def self_backrot(pt, gi, y):
                # xre1 (uncorrected) = c*yre - s*yim
                t5 = p1.tile([128, GR], bf16, tag="t5", bufs=2)
                t6 = p1.tile([128, GR], bf16, tag="t6", bufs=2)
                x1u = x1r.tile(
                    [128, GR], bf16, name=f"x1u_{pt}_{gi}", tag=f"x1u_{pt}_{gi}"
                )
                V.tensor_tensor(t5[:], y[:, 0:GR], ctab_s(pt), op=OP.mult)
                G.tensor_tensor(t6[:], y[:, GR : 2 * GR], stab_s(pt), op=OP.mult)
                V.tensor_tensor(x1u[:], t5[:], t6[:], op=OP.subtract)
                x1res[(pt, gi)] = x1u

            for gi in range(NG):
                c0 = gi * GR
                ut_t = ut0 if gi == 0 else ut_rest[gi - 1]
                ys = []
                for pt in range(PTP):
                    pc = slice(pt * 128, (pt + 1) * 128)
                    bb = p1.tile([128, 2 * GR], bf16, tag="bb", bufs=3)
                    pre = ps1.tile([128, GR], f32, tag="bure")
                    for k in range(KH):
                        nc.tensor.matmul(
                            pre[:], bt[:, k * P + pt * 128 : k * P + (pt + 1) * 128],
                            ut_t[:, k * GR : (k + 1) * GR],
                            start=(k == 0), stop=(k == KH - 1),
                        )
                    A.copy(bb[:, 0:GR], pre[:])
                    pim = ps1.tile([128, GR], f32, tag="buim")
                    for k in range(KH):
                        nc.tensor.matmul(
                            pim[:], bt[:, (KH + k) * P + pt * 128 : (KH + k) * P + (pt + 1) * 128],
                            ut_t[:, k * GR : (k + 1) * GR],
                            start=(k == 0), stop=(k == KH - 1),
                        )
                    A.copy(bb[:, GR : 2 * GR], pim[:])
                    bur = bb[:, 0:GR]
                    bui = bb[:, GR : 2 * GR]
                    # rotation: wre = c*bur + s*bui ; wim = c*bui - s*bur
                    t1 = p1.tile([128, GR], bf16, tag="t1", bufs=2)
                    t2 = p1.tile([128, GR], bf16, tag="t2", bufs=2)
                    t3 = p1.tile([128, GR], bf16, tag="t3", bufs=2)
                    t4 = p1.tile([128, GR], bf16, tag="t4", bufs=2)
                    w = p1.tile([128, 2 * GR], bf16, tag="w", bufs=3)
                    wre = w[:, 0:GR]
                    wim = w[:, GR : 2 * GR]
                    V.tensor_tensor(t1[:], bur, ctab_s(pt), op=OP.mult)
                    G.tensor_tensor(t2[:], bui, stab_s(pt), op=OP.mult)
                    V.tensor_tensor(wre, t1[:], t2[:], op=OP.add)
                    G.tensor_tensor(t3[:], bui, ctab_s(pt), op=OP.mult)
                    G.tensor_tensor(t4[:], bur, stab_s(pt), op=OP.mult)
                    G.tensor_tensor(wim, t3[:], t4[:], op=OP.subtract)
                    nc.sync.dma_start(w_sp[pt][:, 2 * c0 : 2 * c0 + 2 * GR], w[:])
                    # scans
                    y = p1.tile([128, 2 * GR], bf16, tag="y", bufs=10)
                    yre = y[:, 0:GR]
                    yim = y[:, GR : 2 * GR]
                    mb = sm(pt, SM_MVEC).broadcast_to((128, T))
                    V.tensor_tensor_scan(
                        yre, mb, wre, i1re[pt][:, 0:1], op0=OP.mult, op1=OP.add
                    )
                    V.tensor_tensor_scan(
                        yim, mb, wim, i1im[pt][:, 0:1], op0=OP.mult, op1=OP.add
                    )
                    if gi == NG - 1:
                        # defer back-rotation: nothing may delay the end
                        # states that feed the collective
                        end_state(V, yre, yim, pt, p1, 0)
                        ys.append(y)
                        continue
                    subchunk_glue(G, yre, yim, i1re[pt], i1im[pt], pt, p1)
                    self_backrot(pt, gi, y)

                if gi == NG - 1:
                    for pt in range(PTP):
                        self_backrot(pt, gi, ys[pt])

        _ph1es.close()

        # low-rank weights: loaded while the first collective runs
        _fdees = ExitStack()
        fde = _fdees.enter_context(tc.tile_pool(name="fde", bufs=1))
        ft = fde.tile([128, PTP * R], f32r, name="ft", tag="ft")
        nc.sync.dma_start(ft[:], io["FT"].ap())
        dw = fde.tile([128, KR * R], f32r, name="dw", tag="dw")
        nc.sync.dma_start(dw[:], io["DW"].ap())
        et = fde.tile([128, KR * P], f32r, name="et", tag="et")
        nc.sync.dma_start(et[:], io["ET"].ap())

        # ==============================================================
        # carry exchange helper
        # ==============================================================
        def exchange(exi):
            nc.gpsimd.collective_compute(
                "AllGather",
                mybir.AluOpType.bypass,
                replica_groups=[list(range(NCORES))],
                ins=[xe_in[exi].opt()],
                outs=[xe_out[exi].opt()],
            )
            # vectorized across all pt blocks: one gather DMA + wide ops
            xga = glue.tile([128, NCORES, 2 * PTP], f32, tag="xga")
            nc.sync.dma_start(
                xga[:],
                xe_out[exi].rearrange("(r p) w -> p r w", p=128),
            )
            xv = xga[:].rearrange("p r (q c) -> p q r c", c=2)
            smv = smalls[:].rearrange("p (q w) -> p q w", w=SM_W)
            cre_all = smv[:, :, SM_CRE : SM_CRE + NCORES]
            cim_all = smv[:, :, SM_CIM : SM_CIM + NCORES]
            lpr_all = smv[:, :, SM_LPR : SM_LPR + NSUB]
            lpi_all = smv[:, :, SM_LPI : SM_LPI + NSUB]
            xer = xv[:, :, :, 0]
            xei = xv[:, :, :, 1]
            m1 = glue.tile([128, PTP, NCORES], f32, tag="m1")
            m2 = glue.tile([128, PTP, NCORES], f32, tag="m2")
            m3 = glue.tile([128, PTP, NCORES], f32, tag="m3")
            V.tensor_tensor(m1[:], cre_all, xer, op=OP.mult)
            V.tensor_tensor(m2[:], cim_all, xei, op=OP.mult)
            V.tensor_tensor(m3[:], m1[:], m2[:], op=OP.subtract)
            V.tensor_reduce(greA_t[exi][:], m3[:], axis=mybir.AxisListType.X, op=OP.add)
            V.tensor_tensor(m1[:], cre_all, xei, op=OP.mult)
            V.tensor_tensor(m2[:], cim_all, xer, op=OP.mult)
            V.tensor_tensor(m3[:], m1[:], m2[:], op=OP.add)
            V.tensor_reduce(gimA_t[exi][:], m3[:], axis=mybir.AxisListType.X, op=OP.add)
            # G_tau = lam^{tau*T+1} * g
            gre_bc = greA_t[exi][:].broadcast_to((128, PTP, NSUB))
            gim_bc = gimA_t[exi][:].broadcast_to((128, PTP, NSUB))
            a = glue.tile([128, PTP, NSUB], f32, tag="Ga")
            b = glue.tile([128, PTP, NSUB], f32, tag="Gb")
            V.tensor_tensor(a[:], lpr_all, gre_bc, op=OP.mult)
            V.tensor_tensor(b[:], lpi_all, gim_bc, op=OP.mult)
            V.tensor_tensor(GreA_t[exi][:], a[:], b[:], op=OP.subtract)
            V.tensor_tensor(a[:], lpr_all, gim_bc, op=OP.mult)
            V.tensor_tensor(b[:], lpi_all, gre_bc, op=OP.mult)
            V.tensor_tensor(GimA_t[exi][:], a[:], b[:], op=OP.add)
            if exi == 0:
                V.tensor_scalar_mul(nGimA_t[:], GimA_t[exi][:], -1.0)

        # ==============================================================
        # PHASE 2: correction+shift, low-rank path, scan 2
        # ==============================================================
        with (
            tc.tile_pool(name="p2", bufs=2) as p2,
            tc.tile_pool(name="ps2", bufs=4, space="PSUM") as ps2,
        ):
            # prefetch tau=0 w reloads so they transfer during the collective
            wrl_t = {}
            for pt in range(PTP):
                wrl = p2.tile([128, 2 * T], bf16, tag="wrl", bufs=12)
                nc.sync.dma_start(wrl[:], w_sp[pt][:, 0 : 2 * T])
                wrl_t[(pt, 0)] = wrl

            exchange(0)

            def mctab_s(pt, w=T):
                return mctab[:, pt * T : pt * T + w]

            def mstab_s(pt, w=T):
                return mstab[:, pt * T : pt * T + w]

            for pt in range(PTP):
                V.memset(i2re[pt][:], 0.0)
                V.memset(i2im[pt][:], 0.0)

            for tau in range(NSUB):
                c0 = tau * T
                # prefetch next block's w reloads
                if tau + 1 < NSUB:
                    for pt in range(PTP):
                        wrl = p2.tile([128, 2 * T], bf16, tag="wrl", bufs=12)
                        nc.sync.dma_start(
                            wrl[:], w_sp[pt][:, 2 * (c0 + T) : 2 * (c0 + 2 * T)]
                        )
                        wrl_t[(pt, tau + 1)] = wrl
                # --- corrected + shifted x_prev slice per pt ---
                xsh = []
                for pt in range(PTP):
                    gi_r = (tau * T) // GR
                    off = (tau * T) % GR
                    xrl = x1res[(pt, gi_r)][:, off : off + T]
                    x = p2.tile([128, T], f32r, tag="xsh", bufs=8)
                    if tau == 0:
                        V.tensor_copy(x[:, 0:1], greA_t[0][:, pt, :])
                    else:
                        V.tensor_copy(x[:, 0:1], nxb[pt][:])
                    tmpa = p2.tile([128, T - 1], f32, tag="tmpa")
                    V.scalar_tensor_tensor(
                        tmpa[:], mctab_s(pt, T - 1),
                        GreA_t[0][:, pt, tau : tau + 1], xrl[:, 0 : T - 1],
                        op0=OP.mult, op1=OP.add,
                    )
                    V.scalar_tensor_tensor(
                        x[:, 1:T], mstab_s(pt, T - 1),
                        nGimA_t[:, pt, tau : tau + 1], tmpa[:],
                        op0=OP.mult, op1=OP.add,
                    )
                    tmpc = p2.tile([128, 1], f32, tag="tmpc")
                    V.scalar_tensor_tensor(
                        tmpc[:], mctab[:, pt * T + T - 1 : pt * T + T],
                        GreA_t[0][:, pt, tau : tau + 1], xrl[:, T - 1 : T],
                        op0=OP.mult, op1=OP.add,
                    )
                    V.scalar_tensor_tensor(
                        nxb[pt][:], mstab[:, pt * T + T - 1 : pt * T + T],
                        nGimA_t[:, pt, tau : tau + 1], tmpc[:],
                        op0=OP.mult, op1=OP.add,
                    )
                    xsh.append(x)
                # --- q = F @ xsh ---
                q_sb = []
                for mt in range(4):
                    qp = ps2.tile([128, T], f32, tag="qp")
                    mc = slice(mt * 128, (mt + 1) * 128)
                    for k in range(PTP):
                        nc.tensor.matmul(
                            qp[:], ft[:, k * R + mt * 128 : k * R + (mt + 1) * 128], xsh[k][:],
                            start=(k == 0), stop=(k == PTP - 1),
                        )
                    qs = p2.tile([128, T], f32r, tag="qsb", bufs=4)
                    A.copy(qs[:], qp[:])
                    q_sb.append(qs)
                # --- p = Delta^T @ q ---
                p_sb = []
                for mt in range(4):
                    pp = ps2.tile([128, T], f32, tag="qp")
                    for k in range(KR):
                        nc.tensor.matmul(
                            pp[:], dw[:, k * R + mt * 128 : k * R + (mt + 1) * 128], q_sb[k][:],
                            start=(k == 0), stop=(k == KR - 1),
                        )
                    ps_ = p2.tile([128, T], f32r, tag="psb", bufs=4)
                    A.copy(ps_[:], pp[:])
                    p_sb.append(ps_)
                # --- Ep, w2, scan2 per pt (2 groups of 4 to bound PSUM) ---
                for grp in range(2):
                    for pti in range(4):
                        pt = grp * 4 + pti
                        epp = ps2.tile([128, T], f32, tag="ep")
                        for k in range(KR):
                            nc.tensor.matmul(
                                epp[:], et[:, k * P + pt * 128 : k * P + (pt + 1) * 128], p_sb[k][:],
                                start=(k == 0), stop=(k == KR - 1),
                            )
                        wrl = wrl_t.pop((pt, tau))
                        ep_sb = p2.tile([128, T], f32, tag="ep_sb")
                        A.copy(ep_sb[:], epp[:])
                        ta = p2.tile([128, T], f32, tag="ta")
                        tb = p2.tile([128, T], f32, tag="tb")
                        G.tensor_tensor(ta[:], ctab_s(pt), ep_sb[:], op=OP.mult)
                        G.tensor_tensor(tb[:], stab_s(pt), ep_sb[:], op=OP.mult)
                        w2r = p2.tile([128, T], f32, tag="w2r")
                        w2i = p2.tile([128, T], f32, tag="w2i")
                        G.tensor_tensor(w2r[:], wrl[:, 0:T], ta[:], op=OP.add)
                        G.tensor_tensor(w2i[:], wrl[:, T : 2 * T], tb[:], op=OP.subtract)
                        if tau == 0:
                            V.tensor_scalar_mul(w2r[:, 0:1], w2r[:, 0:1], sm(pt, SM_ZCOL))
                            G.tensor_scalar_mul(w2i[:, 0:1], w2i[:, 0:1], sm(pt, SM_ZCOL))
                        y2 = p2.tile([128, 2 * T], bf16, tag="y2", bufs=3)
                        y2r = y2[:, 0:T]
                        y2i = y2[:, T : 2 * T]
                        mb = sm(pt, SM_MVEC).broadcast_to((128, T))
                        V.tensor_tensor_scan(
                            y2r, mb, w2r[:], i2re[pt][:, 0:1], op0=OP.mult, op1=OP.add
                        )
                        V.tensor_tensor_scan(
                            y2i, mb, w2i[:], i2im[pt][:, 0:1], op0=OP.mult, op1=OP.add
                        )
                        nc.sync.dma_start(y2_sp[pt][:, 2 * c0 : 2 * c0 + 2 * T], y2[:])
                        if tau == NSUB - 1:
                            end_state(V, y2r, y2i, pt, p2, 1)
                        else:
                            subchunk_glue(G, y2r, y2i, i2re[pt], i2im[pt], pt, p2)

        _fdees.close()
        _x1es.close()

        # C weights: loaded while the second collective runs
        _ctes = ExitStack()
        ctp = _ctes.enter_context(tc.tile_pool(name="ctp", bufs=1))
        ct = ctp.tile([128, 2 * PTP * H], bf16, name="ct", tag="ct")

        # ==============================================================
        # PHASE 3: y2 correction, rotation back, C projection, output
        # ==============================================================
        with (
            tc.tile_pool(name="p3", bufs=2) as p3,
            tc.tile_pool(name="ps3", bufs=4, space="PSUM") as ps3,
        ):
            def mptab_s(pt):
                return mptab[:, pt * T : (pt + 1) * T]

            # tau=0 activations first (issue instantly: producers done), then
            # C chunks; everything transfers while the collective runs
            u3_t = {}
            yrl_t = {}
            u3 = p3.tile([128, KH * T], bf16, tag="u3", bufs=2)
            nc.sync.dma_start(
                u3[:].rearrange("p (k t) -> p k t", t=T), io["uT"].ap()[:, :, 0:T]
            )
            u3_t[0] = u3
            for pt in range(PTP):
                yrl = p3.tile([128, 2 * T], bf16, tag="yrl", bufs=12)
                nc.sync.dma_start(yrl[:], y2_sp[pt][:, 0 : 2 * T])
                yrl_t[(pt, 0)] = yrl
            for ch in range(4):
                w4 = PTP * H // 2
                nc.sync.dma_start(
                    ct[:, ch * w4 : (ch + 1) * w4], io["CT"].ap()[:, ch * w4 : (ch + 1) * w4]
                )

            exchange(1)

            for tau in range(NSUB):
                c0 = tau * T
                if tau + 1 < NSUB:
                    u3 = p3.tile([128, KH * T], bf16, tag="u3", bufs=2)
                    nc.sync.dma_start(
                        u3[:].rearrange("p (k t) -> p k t", t=T),
                        io["uT"].ap()[:, :, c0 + T : c0 + 2 * T],
                    )
                    u3_t[tau + 1] = u3
                    for pt in range(PTP):
                        yrl = p3.tile([128, 2 * T], bf16, tag="yrl", bufs=12)
                        nc.sync.dma_start(
                            yrl[:], y2_sp[pt][:, 2 * (c0 + T) : 2 * (c0 + 2 * T)]
                        )
                        yrl_t[(pt, tau + 1)] = yrl
                u3 = u3_t.pop(tau)
                xre2, xim2 = [], []
                for pt in range(PTP):
                    yrl = yrl_t.pop((pt, tau))
                    yrc = p3.tile([128, T], f32, tag="yrc")
                    yic = p3.tile([128, T], f32, tag="yic")
                    V.scalar_tensor_tensor(
                        yrc[:], mptab_s(pt), GreA_t[1][:, pt, tau : tau + 1], yrl[:, 0:T],
                        op0=OP.mult, op1=OP.add,
                    )
                    V.scalar_tensor_tensor(
                        yic[:], mptab_s(pt), GimA_t[1][:, pt, tau : tau + 1], yrl[:, T : 2 * T],
                        op0=OP.mult, op1=OP.add,
                    )
                    u1 = p3.tile([128, T], f32, tag="u1")
                    u2 = p3.tile([128, T], f32, tag="u2")
                    u3r = p3.tile([128, T], f32, tag="u3r")
                    u4 = p3.tile([128, T], f32, tag="u4")
                    xr = p3.tile([128, T], bf16, tag="xr", bufs=10)
                    xi = p3.tile([128, T], bf16, tag="xi", bufs=10)
                    G.tensor_tensor(u1[:], ctab_s(pt), yrc[:], op=OP.mult)
                    G.tensor_tensor(u2[:], stab_s(pt), yic[:], op=OP.mult)
                    G.tensor_tensor(xr[:], u1[:], u2[:], op=OP.subtract)
                    G.tensor_tensor(u3r[:], stab_s(pt), yrc[:], op=OP.mult)
                    V.tensor_tensor(u4[:], ctab_s(pt), yic[:], op=OP.mult)
                    G.tensor_tensor(xi[:], u3r[:], u4[:], op=OP.add)
                    xre2.append(xr)
                    xim2.append(xi)
                ostage = p3.tile([128, HT * T], f32, tag="ostage", bufs=2)
                for hb in range(HT):
                    op_ = ps3.tile([128, T], f32, tag="o")
                    for k in range(PTP):
                        nc.tensor.matmul(
                            op_[:], ct[:, k * H + hb * 128 : k * H + (hb + 1) * 128], xre2[k][:],
                            start=(k == 0), stop=False,
                        )
                    for k in range(PTP):
                        nc.tensor.matmul(
                            op_[:], ct[:, (PTP + k) * H + hb * 128 : (PTP + k) * H + (hb + 1) * 128], xim2[k][:],
                            start=False, stop=(k == PTP - 1),
                        )
                    V.scalar_tensor_tensor(
                        ostage[:, hb * T : (hb + 1) * T],
                        u3[:, hb * T : (hb + 1) * T], dvec[:, hb : hb + 1], op_[:],
                        op0=OP.mult, op1=OP.add,
                    )
                nc.sync.dma_start(
                    io["outT"].ap()[:, :, c0 : c0 + T],
                    ostage[:].rearrange("p (h t) -> p h t", t=T),
                )

        _ctes.close()


def build_program(cfg):
    import concourse.bacc as bacc
    import concourse.mybir as mybir
    import concourse.tile as tile

    f32 = mybir.dt.float32
    f32r = mybir.dt.float32r
    bf16 = mybir.dt.bfloat16
    T = cfg["T"]
    S = cfg["L"] // NCORES
    KH = H // 128
    PTP = P // 128
    KR = R // 128
    HT = H // 128

    nc = bacc.Bacc(
        "TRN2", target_bir_lowering=False, debug=False, num_devices=NCORES
    )
    io = {}
    ins = [
        ("uT", (128, KH, S), bf16),
        ("BT", (128, 2 * KH * P), bf16),
        ("CT", (128, 2 * PTP * H), bf16),
        ("FT", (128, PTP * R), f32r),
        ("DW", (128, KR * R), f32r),
        ("ET", (128, KR * P), f32r),
        ("Dvec", (128, HT), f32),
        ("ctab", (128, PTP * T), bf16),
        ("stab", (128, PTP * T), bf16),
        ("mctab", (128, PTP * T), bf16),
        ("mstab", (128, PTP * T), bf16),
        ("mptab", (128, PTP * T), bf16),
        ("smalls", (128, PTP * SM_W), f32),
    ]
    for name, shape, dt_ in ins:
        io[name] = nc.dram_tensor(name, list(shape), dt_, kind="ExternalInput")
    io["outT"] = nc.dram_tensor("outT", [128, HT, S], f32, kind="ExternalOutput")

    with tile.TileContext(nc) as tc:
        _emit(nc, tc, io, cfg)
    nc.compile()
    return nc


# ======================================================================
# host side
# ======================================================================

def make_tables(lam_re, lam_im, cfg):
    T = cfg["T"]
    S = cfg["L"] // NCORES
    NSUB = S // T
    PTP = P // 128
    f32 = np.float32
    bfl = ml_dtypes.bfloat16
    lam = lam_re.astype(np.float64) + 1j * lam_im.astype(np.float64)
    mag = np.abs(lam)
    th = np.angle(lam)
    k = np.arange(T)
    ctab = np.cos(np.outer(th, k))          # (P, T) f64
    stab = np.sin(np.outer(th, k))
    mptab = mag[:, None] ** k[None, :]
    mctab = mptab * ctab
    mstab = mptab * stab
    tau = np.arange(NSUB)
    lpow = lam[:, None] ** (tau[None, :] * T + 1)

    def tile_pt(arr, dtype):
        # (P, W) -> (128, PTP*W): per-pt blocks of columns
        W = arr.shape[1]
        out = np.empty((128, PTP * W), dtype)
        for pt in range(PTP):
            out[:, pt * W : (pt + 1) * W] = arr[pt * 128 : (pt + 1) * 128]
        return out

    tabs = dict(
        ctab=tile_pt(ctab, bfl), stab=tile_pt(stab, bfl),
        mptab=tile_pt(mptab, bfl),
        mctab=tile_pt(mctab, bfl), mstab=tile_pt(mstab, bfl),
    )
    # smalls blob
    sm_full = np.zeros((P, SM_W), f32)
    sm_full[:, SM_MVEC] = mag
    sm_full[:, SM_COST] = np.cos(T * th)
    sm_full[:, SM_SINT] = np.sin(T * th)
    sm_full[:, SM_COST1] = np.cos((T - 1) * th)
    sm_full[:, SM_SINT1] = np.sin((T - 1) * th)
    sm_full[:, SM_LPR : SM_LPR + NSUB] = np.real(lpow)
    sm_full[:, SM_LPI : SM_LPI + NSUB] = np.imag(lpow)
    coefre = np.zeros((NCORES, P, NCORES), f32)
    coefim = np.zeros((NCORES, P, NCORES), f32)
    for m in range(NCORES):
        for j in range(m):
            v = lam ** (S * (m - 1 - j))
            coefre[m, :, j] = np.real(v)
            coefim[m, :, j] = np.imag(v)
    return tabs, sm_full, coefre, coefim


def make_in_maps(inputs, cfg):
    f32 = np.float32
    bfl = ml_dtypes.bfloat16
    Lc = cfg["L"]
    S = Lc // NCORES
    KH = H // 128
    PTP = P // 128
    KR = R // 128
    HT = H // 128
    u = np.ascontiguousarray(np.asarray(inputs["input_sequence"], f32)[:Lc])
    tabs, sm_full, coefre, coefim = make_tables(
        np.asarray(inputs["Lambda_re"]), np.asarray(inputs["Lambda_im"]), cfg
    )

    def pack_k(mat, nk, width, dtype):
        # mat: (nk*128, width) -> (128, nk*width)
        out = np.empty((128, nk * width), dtype)
        for k in range(nk):
            out[:, k * width : (k + 1) * width] = mat[k * 128 : (k + 1) * 128]
        return out

    BTre = np.asarray(inputs["B_re"], f32).T    # (H, P)
    BTim = np.asarray(inputs["B_im"], f32).T
    BT = np.concatenate(
        [pack_k(BTre, KH, P, bfl), pack_k(BTim, KH, P, bfl)], axis=1
    )
    CreT = np.asarray(inputs["C_re"], f32).T    # (P, H)
    nCimT = -np.asarray(inputs["C_im"], f32).T
    CT = np.concatenate(
        [pack_k(CreT, PTP, H, bfl), pack_k(nCimT, PTP, H, bfl)], axis=1
    )
    FT = pack_k(np.asarray(inputs["F"], f32).T, PTP, R, f32)       # (P,R) tiles
    DW = pack_k(np.asarray(inputs["Delta"], f32), KR, R, f32)      # (R,R) rows
    ET = pack_k(np.asarray(inputs["E"], f32).T, KR, P, f32)        # (R,P) tiles
    Dvec = pack_k(np.asarray(inputs["D"], f32)[:, None], HT, 1, f32)  # (128, HT)

    in_maps = []
    for m in range(NCORES):
        sm_m = sm_full.copy()
        sm_m[:, SM_ZCOL] = 0.0 if m == 0 else 1.0
        sm_m[:, SM_CRE : SM_CRE + NCORES] = coefre[m]
        sm_m[:, SM_CIM : SM_CIM + NCORES] = coefim[m]
        # tile smalls per pt
        sm_t = np.empty((128, PTP * SM_W), f32)
        for pt in range(PTP):
            sm_t[:, pt * SM_W : (pt + 1) * SM_W] = sm_m[pt * 128 : (pt + 1) * 128]
        uT = np.ascontiguousarray(u[m * S : (m + 1) * S, :].T)  # (H, S)
        uT_t = np.empty((128, KH, S), bfl)
        for k in range(KH):
            uT_t[:, k, :] = uT[k * 128 : (k + 1) * 128]
        im = dict(
            uT=uT_t, BT=BT, CT=CT, FT=FT, DW=DW, ET=ET, Dvec=Dvec,
            smalls=sm_t, **tabs,
        )
        in_maps.append(im)
    return in_maps


def assemble_output(results, cfg):
    Lc = cfg["L"]
    S = Lc // NCORES
    out = np.empty((Lc, H), np.float32)
    for m in range(NCORES):
        # outT: (128, HT, S) -> (S, H)
        o = results[m]["outT"]
        out[m * S : (m + 1) * S, :] = o.transpose(2, 1, 0).reshape(S, H)
    out[0, :] = 0.0
    return out


def get_program(cfg_key="full"):
    if cfg_key not in _PROG_CACHE:
        _PROG_CACHE[cfg_key] = build_program(CFG_FULL)
    return _PROG_CACHE[cfg_key]


def run(inputs, trace=False, **kw):
    from concourse import bass_utils

    nc = get_program()
    in_maps = make_in_maps(inputs, CFG_FULL)
    res = bass_utils.run_bass_kernel_spmd(
        nc, in_maps, core_ids=list(range(NCORES)), trace=trace, **kw
    )
    return assemble_output(res.results, CFG_FULL), res


def kernel(**inputs):
    out, _ = run(inputs)
    return out


# revision 37
# speedup vs baseline: 1.0089x; 1.0089x over previous
"""Extended S5 SSM on 8 Trainium2 NeuronCores (Bass/Tile).

Algorithm (sequence-parallel, L sharded across 8 cores, feature-on-partition
layout everywhere so no on-device transposes are needed):

  pass 1:  Bu = B @ u^T               (PE, bf16)
           complex diagonal scan over L via rotation factorization:
             x_k = lam*x_{k-1} + b_k,  lam = m*e^{i th}
             y_k = e^{-ik th} x_k  =>  y_k = m*y_{k-1} + e^{-ik th} b_k
           i.e. two independent REAL first-order scans (tensor_tensor_scan)
           per P-lane after a complex "rotation" of the input.  Subchunks of
           T steps chain through tiny per-partition init rotations; cores
           chain through one 8 KB AllGather of end-states (the carry), with
           the homogeneous correction  y += m^k * G  applied afterwards.
  low rank: q = F x_prev, p = q Delta, Ep = E p  (PE)
  pass 2:  scan of (Bu + Ep-rotated), same machinery, second AllGather
  out:     out^T = Cre xre2 - Cim xim2 + D*u^T   (PE + fused stt)

v2: all host inputs packed so each parameter family loads in ONE DMA
(HWDGE issue time dominated the baseline); bf16 weights/activations for
the big matmuls; paired [re|im] spill tiles; packed output stores;
phase-2/3 parameters prefetched during earlier phases.
"""

import sys

import numpy as np

for _p in ("/opt/trn_rl_repo", "/root/.axon_site/_ro/trn_rl_repo"):
    if _p not in sys.path:
        sys.path.append(_p)

try:
    import ml_dtypes
except ImportError:
    ml_dtypes = None

# ---- problem geometry (hardcoded; harness contract) ----
L, H, P, R = 16384, 1024, 1024, 512
NCORES = 8

CFG_FULL = dict(L=16384, T=512, GR=512)

_PROG_CACHE = {}

# smalls blob per-pt column layout (f32)
SM_W = 30
SM_MVEC, SM_COST, SM_SINT, SM_COST1, SM_SINT1, SM_ZCOL = 0, 1, 2, 3, 4, 5
SM_LPR, SM_LPI = 6, 10          # 4 wide each (NSUB)
SM_CRE, SM_CIM = 14, 22         # 8 wide each (NCORES)


# ======================================================================
# device program
# ======================================================================

def _emit(nc, tc, io, cfg):
    import concourse.mybir as mybir

    f32 = mybir.dt.float32
    f32r = mybir.dt.float32r
    bf16 = mybir.dt.bfloat16
    OP = mybir.AluOpType

    T = cfg["T"]
    GR = cfg["GR"]
    S = cfg["L"] // NCORES
    NSUB = S // T
    NG = S // GR
    KH = H // 128
    PTP = P // 128
    KR = R // 128
    HT = H // 128

    V = nc.vector
    G = nc.gpsimd
    A = nc.scalar

    from contextlib import ExitStack

    _ph1es = ExitStack()
    with (
        tc.tile_pool(name="persist", bufs=1) as persist,
        tc.tile_pool(name="glue", bufs=1) as glue,
        tc.tile_pool(name="dram", bufs=1, space="DRAM") as dram,
    ):
        # x1 results live until end of phase 2; opened first for LIFO pool order
        _x1es = ExitStack()
        x1r = _x1es.enter_context(tc.tile_pool(name="x1r", bufs=1))

        # phase-1-lifetime pool (B weights + u tiles; freed before C prefetch)
        ph1w = _ph1es.enter_context(tc.tile_pool(name="ph1w", bufs=1))

        # ---------- critical-path first loads: u group 0, then B chunks ----------
        ut0 = ph1w.tile([128, KH * GR], bf16, name="ut0", tag="ut0")
        nc.sync.dma_start(
            ut0[:].rearrange("p (k t) -> p k t", t=GR),
            io["uT"].ap()[:, :, 0:GR],
        )
        bt = ph1w.tile([128, 2 * KH * P], bf16, name="bt", tag="bt")
        bt_cuts = [0, 2 * P, 8 * P, 10 * P, 16 * P]
        for ch in range(4):
            nc.sync.dma_start(
                bt[:, bt_cuts[ch] : bt_cuts[ch + 1]],
                io["BT"].ap()[:, bt_cuts[ch] : bt_cuts[ch + 1]],
            )

        # ---------- tables (single DMA each) ----------
        ctab = persist.tile([128, PTP * T], bf16, name="ctab", tag="ctab")
        nc.sync.dma_start(ctab[:], io["ctab"].ap())
        stab = persist.tile([128, PTP * T], bf16, name="stab", tag="stab")
        nc.sync.dma_start(stab[:], io["stab"].ap())
        smalls = persist.tile([128, PTP * SM_W], f32, name="smalls", tag="smalls")
        nc.sync.dma_start(smalls[:], io["smalls"].ap())

        def sm(pt, c0, w=1):
            return smalls[:, pt * SM_W + c0 : pt * SM_W + c0 + w]

        def ctab_s(pt):
            return ctab[:, pt * T : (pt + 1) * T]

        def stab_s(pt):
            return stab[:, pt * T : (pt + 1) * T]

        # remaining u groups (phase 1 streams them; emitted early, deps none)
        ut_rest = []
        for gi in range(1, NG):
            t = ph1w.tile([128, KH * GR], bf16, name=f"ut{gi}", tag=f"ut{gi}")
            nc.sync.dma_start(
                t[:].rearrange("p (k t) -> p k t", t=GR),
                io["uT"].ap()[:, :, gi * GR : (gi + 1) * GR],
            )
            ut_rest.append(t)

        # ---------- phase-2/3 parameter prefetch (issued early; consumed later) ----------
        mctab = persist.tile([128, PTP * T], bf16, name="mctab", tag="mctab")
        nc.sync.dma_start(mctab[:], io["mctab"].ap())
        mstab = persist.tile([128, PTP * T], bf16, name="mstab", tag="mstab")
        nc.sync.dma_start(mstab[:], io["mstab"].ap())
        mptab = persist.tile([128, PTP * T], bf16, name="mptab", tag="mptab")
        nc.sync.dma_start(mptab[:], io["mptab"].ap())
        dvec = persist.tile([128, HT], f32, name="dvec", tag="dvec")
        nc.sync.dma_start(dvec[:], io["Dvec"].ap())

        # ---------- DRAM scratch ----------
        w_sp = [dram.tile([128, 2 * S], bf16, name=f"w_sp{pt}", tag=f"w_sp{pt}") for pt in range(PTP)]
        y2_sp = [dram.tile([128, 2 * S], bf16, name=f"y2_sp{pt}", tag=f"y2_sp{pt}") for pt in range(PTP)]
        xe_in = [dram.tile([128, 2 * PTP], bf16, name=f"xe_in{e}", tag=f"xe_in{e}") for e in range(2)]
        xe_out = [
            dram.tile([NCORES * 128, 2 * PTP], bf16, addr_space="Shared", name=f"xe_out{e}", tag=f"xe_out{e}")
            for e in range(2)
        ]

        # ---------- persistent small state ----------
        i1re = [glue.tile([128, 1], f32, name=f"i1re{pt}", tag=f"i1re{pt}") for pt in range(PTP)]
        i1im = [glue.tile([128, 1], f32, name=f"i1im{pt}", tag=f"i1im{pt}") for pt in range(PTP)]
        i2re = [glue.tile([128, 1], f32, name=f"i2re{pt}", tag=f"i2re{pt}") for pt in range(PTP)]
        i2im = [glue.tile([128, 1], f32, name=f"i2im{pt}", tag=f"i2im{pt}") for pt in range(PTP)]
        nxb = [glue.tile([128, 1], f32, name=f"nxb{pt}", tag=f"nxb{pt}") for pt in range(PTP)]
        es_pk = [glue.tile([128, 2 * PTP], bf16, name=f"es_pk{e}", tag=f"es_pk{e}") for e in range(2)]
        greA_t = [glue.tile([128, PTP, 1], f32, name=f"g{e}reA", tag=f"g{e}reA") for e in range(2)]
        gimA_t = [glue.tile([128, PTP, 1], f32, name=f"g{e}imA", tag=f"g{e}imA") for e in range(2)]
        GreA_t = [glue.tile([128, PTP, NSUB], f32, name=f"G{e}reA", tag=f"G{e}reA") for e in range(2)]
        GimA_t = [glue.tile([128, PTP, NSUB], f32, name=f"G{e}imA", tag=f"G{e}imA") for e in range(2)]
        nGimA_t = glue.tile([128, PTP, NSUB], f32, name="nGimA", tag="nGimA")

        def subchunk_glue(eng, y_re_ap, y_im_ap, ire, iim, pt, sc_pool):
            # init_{tau+1} = e^{i T th} * y_last  (per-partition rotation)
            yr = y_re_ap[:, T - 1 : T]
            yi = y_im_ap[:, T - 1 : T]
            a = sc_pool.tile([128, 1], f32, tag="ga")
            b = sc_pool.tile([128, 1], f32, tag="gb")
            eng.tensor_scalar_mul(a[:], yr, sm(pt, SM_COST))
            eng.tensor_scalar_mul(b[:], yi, sm(pt, SM_SINT))
            c = sc_pool.tile([128, 1], f32, tag="gc")
            d = sc_pool.tile([128, 1], f32, tag="gd")
            eng.tensor_scalar_mul(c[:], yr, sm(pt, SM_SINT))
            eng.tensor_scalar_mul(d[:], yi, sm(pt, SM_COST))
            eng.tensor_tensor(ire[:], a[:], b[:], op=OP.subtract)
            eng.tensor_tensor(iim[:], c[:], d[:], op=OP.add)

        def end_state(eng, y_re_ap, y_im_ap, pt, sc_pool, exi):
            # x_end = e^{i (T-1) th} * y_last -> packed (re,im) per pt
            yr = y_re_ap[:, T - 1 : T]
            yi = y_im_ap[:, T - 1 : T]
            a = sc_pool.tile([128, 1], f32, tag="ea")
            b = sc_pool.tile([128, 1], f32, tag="eb")
            eng.tensor_scalar_mul(a[:], yr, sm(pt, SM_COST1))
            eng.tensor_scalar_mul(b[:], yi, sm(pt, SM_SINT1))
            pk = es_pk[exi]
            eng.tensor_tensor(pk[:, 2 * pt : 2 * pt + 1], a[:], b[:], op=OP.subtract)
            eng.tensor_scalar_mul(a[:], yr, sm(pt, SM_SINT1))
            eng.tensor_scalar_mul(b[:], yi, sm(pt, SM_COST1))
            eng.tensor_tensor(pk[:, 2 * pt + 1 : 2 * pt + 2], a[:], b[:], op=OP.add)
            if pt == PTP - 1:
                nc.sync.dma_start(xe_in[exi][:, :], pk[:])

        # ==============================================================
        # PHASE 1: Bu matmuls, rotation, local scan, spills, end states
        # ==============================================================
        x1res = {}
        with (
            tc.tile_pool(name="p1", bufs=2) as p1,
            tc.tile_pool(name="ps1", bufs=4, space="PSUM") as ps1,
        ):
            for pt in range(PTP):
                V.memset(i1re[pt][:], 0.0)
                V.memset(i1im[pt][:], 0.0)

            def self_backrot(pt, gi, y):
                # xre1 (uncorrected) = c*yre - s*yim
                t5 = p1.tile([128, GR], bf16, tag="t5", bufs=2)
                t6 = p1.tile([128, GR], bf16, tag="t6", bufs=2)
                x1u = x1r.tile(
                    [128, GR], bf16, name=f"x1u_{pt}_{gi}", tag=f"x1u_{pt}_{gi}"
                )
                V.tensor_tensor(t5[:], y[:, 0:GR], ctab_s(pt), op=OP.mult)
                G.tensor_tensor(t6[:], y[:, GR : 2 * GR], stab_s(pt), op=OP.mult)
                V.tensor_tensor(x1u[:], t5[:], t6[:], op=OP.subtract)
                x1res[(pt, gi)] = x1u

            for gi in range(NG):
                c0 = gi * GR
                ut_t = ut0 if gi == 0 else ut_rest[gi - 1]
                ys = []
                for pt in range(PTP):
                    pc = slice(pt * 128, (pt + 1) * 128)
                    bb = p1.tile([128, 2 * GR], bf16, tag="bb", bufs=3)
                    pre = ps1.tile([128, GR], f32, tag="bure")
                    for k in range(KH):
                        nc.tensor.matmul(
                            pre[:], bt[:, k * P + pt * 128 : k * P + (pt + 1) * 128],
                            ut_t[:, k * GR : (k + 1) * GR],
                            start=(k == 0), stop=(k == KH - 1),
                        )
                    A.copy(bb[:, 0:GR], pre[:])
                    pim = ps1.tile([128, GR], f32, tag="buim")
                    for k in range(KH):
                        nc.tensor.matmul(
                            pim[:], bt[:, (KH + k) * P + pt * 128 : (KH + k) * P + (pt + 1) * 128],
                            ut_t[:, k * GR : (k + 1) * GR],
                            start=(k == 0), stop=(k == KH - 1),
                        )
                    A.copy(bb[:, GR : 2 * GR], pim[:])
                    bur = bb[:, 0:GR]
                    bui = bb[:, GR : 2 * GR]
                    # rotation: wre = c*bur + s*bui ; wim = c*bui - s*bur
                    t1 = p1.tile([128, GR], bf16, tag="t1", bufs=2)
                    t2 = p1.tile([128, GR], bf16, tag="t2", bufs=2)
                    t3 = p1.tile([128, GR], bf16, tag="t3", bufs=2)
                    t4 = p1.tile([128, GR], bf16, tag="t4", bufs=2)
                    w = p1.tile([128, 2 * GR], bf16, tag="w", bufs=3)
                    wre = w[:, 0:GR]
                    wim = w[:, GR : 2 * GR]
                    V.tensor_tensor(t1[:], bur, ctab_s(pt), op=OP.mult)
                    G.tensor_tensor(t2[:], bui, stab_s(pt), op=OP.mult)
                    V.tensor_tensor(wre, t1[:], t2[:], op=OP.add)
                    G.tensor_tensor(t3[:], bui, ctab_s(pt), op=OP.mult)
                    G.tensor_tensor(t4[:], bur, stab_s(pt), op=OP.mult)
                    G.tensor_tensor(wim, t3[:], t4[:], op=OP.subtract)
                    nc.sync.dma_start(w_sp[pt][:, 2 * c0 : 2 * c0 + 2 * GR], w[:])
                    # scans
                    y = p1.tile([128, 2 * GR], bf16, tag="y", bufs=10)
                    yre = y[:, 0:GR]
                    yim = y[:, GR : 2 * GR]
                    mb = sm(pt, SM_MVEC).broadcast_to((128, T))
                    V.tensor_tensor_scan(
                        yre, mb, wre, i1re[pt][:, 0:1], op0=OP.mult, op1=OP.add
                    )
                    V.tensor_tensor_scan(
                        yim, mb, wim, i1im[pt][:, 0:1], op0=OP.mult, op1=OP.add
                    )
                    if gi == NG - 1:
                        # defer back-rotation: nothing may delay the end
                        # states that feed the collective
                        end_state(V, yre, yim, pt, p1, 0)
                        ys.append(y)
                        continue
                    subchunk_glue(G, yre, yim, i1re[pt], i1im[pt], pt, p1)
                    self_backrot(pt, gi, y)

                if gi == NG - 1:
                    for pt in range(PTP):
                        self_backrot(pt, gi, ys[pt])

        _ph1es.close()

        # low-rank weights: loaded while the first collective runs
        _fdees = ExitStack()
        fde = _fdees.enter_context(tc.tile_pool(name="fde", bufs=1))
        ft = fde.tile([128, PTP * R], f32r, name="ft", tag="ft")
        nc.sync.dma_start(ft[:], io["FT"].ap())
        dw = fde.tile([128, KR * R], f32r, name="dw", tag="dw")
        nc.sync.dma_start(dw[:], io["DW"].ap())
        et = fde.tile([128, KR * P], f32r, name="et", tag="et")
        nc.sync.dma_start(et[:], io["ET"].ap())

        # ==============================================================
        # carry exchange helper
        # ==============================================================
        def exchange(exi):
            nc.gpsimd.collective_compute(
                "AllGather",
                mybir.AluOpType.bypass,
                replica_groups=[list(range(NCORES))],
                ins=[xe_in[exi].opt()],
                outs=[xe_out[exi].opt()],
            )
            # vectorized across all pt blocks: one gather DMA + wide ops
            xga = glue.tile([128, NCORES, 2 * PTP], bf16, tag="xga")
            nc.sync.dma_start(
                xga[:],
                xe_out[exi].rearrange("(r p) w -> p r w", p=128),
            )
            xv = xga[:].rearrange("p r (q c) -> p q r c", c=2)
            smv = smalls[:].rearrange("p (q w) -> p q w", w=SM_W)
            cre_all = smv[:, :, SM_CRE : SM_CRE + NCORES]
            cim_all = smv[:, :, SM_CIM : SM_CIM + NCORES]
            lpr_all = smv[:, :, SM_LPR : SM_LPR + NSUB]
            lpi_all = smv[:, :, SM_LPI : SM_LPI + NSUB]
            xer = xv[:, :, :, 0]
            xei = xv[:, :, :, 1]
            m1 = glue.tile([128, PTP, NCORES], f32, tag="m1")
            m2 = glue.tile([128, PTP, NCORES], f32, tag="m2")
            m3 = glue.tile([128, PTP, NCORES], f32, tag="m3")
            V.tensor_tensor(m1[:], cre_all, xer, op=OP.mult)
            V.tensor_tensor(m2[:], cim_all, xei, op=OP.mult)
            V.tensor_tensor(m3[:], m1[:], m2[:], op=OP.subtract)
            V.tensor_reduce(greA_t[exi][:], m3[:], axis=mybir.AxisListType.X, op=OP.add)
            V.tensor_tensor(m1[:], cre_all, xei, op=OP.mult)
            V.tensor_tensor(m2[:], cim_all, xer, op=OP.mult)
            V.tensor_tensor(m3[:], m1[:], m2[:], op=OP.add)
            V.tensor_reduce(gimA_t[exi][:], m3[:], axis=mybir.AxisListType.X, op=OP.add)
            # G_tau = lam^{tau*T+1} * g
            gre_bc = greA_t[exi][:].broadcast_to((128, PTP, NSUB))
            gim_bc = gimA_t[exi][:].broadcast_to((128, PTP, NSUB))
            a = glue.tile([128, PTP, NSUB], f32, tag="Ga")
            b = glue.tile([128, PTP, NSUB], f32, tag="Gb")
            V.tensor_tensor(a[:], lpr_all, gre_bc, op=OP.mult)
            V.tensor_tensor(b[:], lpi_all, gim_bc, op=OP.mult)
            V.tensor_tensor(GreA_t[exi][:], a[:], b[:], op=OP.subtract)
            V.tensor_tensor(a[:], lpr_all, gim_bc, op=OP.mult)
            V.tensor_tensor(b[:], lpi_all, gre_bc, op=OP.mult)
            V.tensor_tensor(GimA_t[exi][:], a[:], b[:], op=OP.add)
            if exi == 0:
                V.tensor_scalar_mul(nGimA_t[:], GimA_t[exi][:], -1.0)

        # ==============================================================
        # PHASE 2: correction+shift, low-rank path, scan 2
        # ==============================================================
        with (
            tc.tile_pool(name="p2", bufs=2) as p2,
            tc.tile_pool(name="ps2", bufs=4, space="PSUM") as ps2,
        ):
            # prefetch tau=0 w reloads so they transfer during the collective
            wrl_t = {}
            for pt in range(PTP):
                wrl = p2.tile([128, 2 * T], bf16, tag="wrl", bufs=12)
                nc.sync.dma_start(wrl[:], w_sp[pt][:, 0 : 2 * T])
                wrl_t[(pt, 0)] = wrl

            exchange(0)

            def mctab_s(pt, w=T):
                return mctab[:, pt * T : pt * T + w]

            def mstab_s(pt, w=T):
                return mstab[:, pt * T : pt * T + w]

            for pt in range(PTP):
                V.memset(i2re[pt][:], 0.0)
                V.memset(i2im[pt][:], 0.0)

            for tau in range(NSUB):
                c0 = tau * T
                # prefetch next block's w reloads
                if tau + 1 < NSUB:
                    for pt in range(PTP):
                        wrl = p2.tile([128, 2 * T], bf16, tag="wrl", bufs=12)
                        nc.sync.dma_start(
                            wrl[:], w_sp[pt][:, 2 * (c0 + T) : 2 * (c0 + 2 * T)]
                        )
                        wrl_t[(pt, tau + 1)] = wrl
                # --- corrected + shifted x_prev slice per pt ---
                xsh = []
                for pt in range(PTP):
                    gi_r = (tau * T) // GR
                    off = (tau * T) % GR
                    xrl = x1res[(pt, gi_r)][:, off : off + T]
                    x = p2.tile([128, T], f32r, tag="xsh", bufs=8)
                    if tau == 0:
                        V.tensor_copy(x[:, 0:1], greA_t[0][:, pt, :])
                    else:
                        V.tensor_copy(x[:, 0:1], nxb[pt][:])
                    tmpa = p2.tile([128, T - 1], f32, tag="tmpa")
                    V.scalar_tensor_tensor(
                        tmpa[:], mctab_s(pt, T - 1),
                        GreA_t[0][:, pt, tau : tau + 1], xrl[:, 0 : T - 1],
                        op0=OP.mult, op1=OP.add,
                    )
                    V.scalar_tensor_tensor(
                        x[:, 1:T], mstab_s(pt, T - 1),
                        nGimA_t[:, pt, tau : tau + 1], tmpa[:],
                        op0=OP.mult, op1=OP.add,
                    )
                    tmpc = p2.tile([128, 1], f32, tag="tmpc")
                    V.scalar_tensor_tensor(
                        tmpc[:], mctab[:, pt * T + T - 1 : pt * T + T],
                        GreA_t[0][:, pt, tau : tau + 1], xrl[:, T - 1 : T],
                        op0=OP.mult, op1=OP.add,
                    )
                    V.scalar_tensor_tensor(
                        nxb[pt][:], mstab[:, pt * T + T - 1 : pt * T + T],
                        nGimA_t[:, pt, tau : tau + 1], tmpc[:],
                        op0=OP.mult, op1=OP.add,
                    )
                    xsh.append(x)
                # --- q = F @ xsh ---
                q_sb = []
                for mt in range(4):
                    qp = ps2.tile([128, T], f32, tag="qp")
                    mc = slice(mt * 128, (mt + 1) * 128)
                    for k in range(PTP):
                        nc.tensor.matmul(
                            qp[:], ft[:, k * R + mt * 128 : k * R + (mt + 1) * 128], xsh[k][:],
                            start=(k == 0), stop=(k == PTP - 1),
                        )
                    qs = p2.tile([128, T], f32r, tag="qsb", bufs=4)
                    A.copy(qs[:], qp[:])
                    q_sb.append(qs)
                # --- p = Delta^T @ q ---
                p_sb = []
                for mt in range(4):
                    pp = ps2.tile([128, T], f32, tag="qp")
                    for k in range(KR):
                        nc.tensor.matmul(
                            pp[:], dw[:, k * R + mt * 128 : k * R + (mt + 1) * 128], q_sb[k][:],
                            start=(k == 0), stop=(k == KR - 1),
                        )
                    ps_ = p2.tile([128, T], f32r, tag="psb", bufs=4)
                    A.copy(ps_[:], pp[:])
                    p_sb.append(ps_)
                # --- Ep, w2, scan2 per pt (2 groups of 4 to bound PSUM) ---
                for grp in range(2):
                    for pti in range(4):
                        pt = grp * 4 + pti
                        epp = ps2.tile([128, T], f32, tag="ep")
                        for k in range(KR):
                            nc.tensor.matmul(
                                epp[:], et[:, k * P + pt * 128 : k * P + (pt + 1) * 128], p_sb[k][:],
                                start=(k == 0), stop=(k == KR - 1),
                            )
                        wrl = wrl_t.pop((pt, tau))
                        ep_sb = p2.tile([128, T], f32, tag="ep_sb")
                        A.copy(ep_sb[:], epp[:])
                        ta = p2.tile([128, T], f32, tag="ta")
                        tb = p2.tile([128, T], f32, tag="tb")
                        G.tensor_tensor(ta[:], ctab_s(pt), ep_sb[:], op=OP.mult)
                        G.tensor_tensor(tb[:], stab_s(pt), ep_sb[:], op=OP.mult)
                        w2r = p2.tile([128, T], f32, tag="w2r")
                        w2i = p2.tile([128, T], f32, tag="w2i")
                        G.tensor_tensor(w2r[:], wrl[:, 0:T], ta[:], op=OP.add)
                        G.tensor_tensor(w2i[:], wrl[:, T : 2 * T], tb[:], op=OP.subtract)
                        if tau == 0:
                            V.tensor_scalar_mul(w2r[:, 0:1], w2r[:, 0:1], sm(pt, SM_ZCOL))
                            G.tensor_scalar_mul(w2i[:, 0:1], w2i[:, 0:1], sm(pt, SM_ZCOL))
                        y2 = p2.tile([128, 2 * T], bf16, tag="y2", bufs=3)
                        y2r = y2[:, 0:T]
                        y2i = y2[:, T : 2 * T]
                        mb = sm(pt, SM_MVEC).broadcast_to((128, T))
                        V.tensor_tensor_scan(
                            y2r, mb, w2r[:], i2re[pt][:, 0:1], op0=OP.mult, op1=OP.add
                        )
                        V.tensor_tensor_scan(
                            y2i, mb, w2i[:], i2im[pt][:, 0:1], op0=OP.mult, op1=OP.add
                        )
                        nc.sync.dma_start(y2_sp[pt][:, 2 * c0 : 2 * c0 + 2 * T], y2[:])
                        if tau == NSUB - 1:
                            end_state(V, y2r, y2i, pt, p2, 1)
                        else:
                            subchunk_glue(G, y2r, y2i, i2re[pt], i2im[pt], pt, p2)

        _fdees.close()
        _x1es.close()

        # C weights: loaded while the second collective runs
        _ctes = ExitStack()
        ctp = _ctes.enter_context(tc.tile_pool(name="ctp", bufs=1))
        ct = ctp.tile([128, 2 * PTP * H], bf16, name="ct", tag="ct")

        # ==============================================================
        # PHASE 3: y2 correction, rotation back, C projection, output
        # ==============================================================
        with (
            tc.tile_pool(name="p3", bufs=2) as p3,
            tc.tile_pool(name="ps3", bufs=4, space="PSUM") as ps3,
        ):
            def mptab_s(pt):
                return mptab[:, pt * T : (pt + 1) * T]

            # tau=0 activations first (issue instantly: producers done), then
            # C chunks; everything transfers while the collective runs
            u3_t = {}
            yrl_t = {}
            u3 = p3.tile([128, KH * T], bf16, tag="u3", bufs=2)
            nc.sync.dma_start(
                u3[:].rearrange("p (k t) -> p k t", t=T), io["uT"].ap()[:, :, 0:T]
            )
            u3_t[0] = u3
            for pt in range(PTP):
                yrl = p3.tile([128, 2 * T], bf16, tag="yrl", bufs=12)
                nc.sync.dma_start(yrl[:], y2_sp[pt][:, 0 : 2 * T])
                yrl_t[(pt, 0)] = yrl
            for ch in range(4):
                w4 = PTP * H // 2
                nc.sync.dma_start(
                    ct[:, ch * w4 : (ch + 1) * w4], io["CT"].ap()[:, ch * w4 : (ch + 1) * w4]
                )

            exchange(1)

            for tau in range(NSUB):
                c0 = tau * T
                if tau + 1 < NSUB:
                    u3 = p3.tile([128, KH * T], bf16, tag="u3", bufs=2)
                    nc.sync.dma_start(
                        u3[:].rearrange("p (k t) -> p k t", t=T),
                        io["uT"].ap()[:, :, c0 + T : c0 + 2 * T],
                    )
                    u3_t[tau + 1] = u3
                    for pt in range(PTP):
                        yrl = p3.tile([128, 2 * T], bf16, tag="yrl", bufs=12)
                        nc.sync.dma_start(
                            yrl[:], y2_sp[pt][:, 2 * (c0 + T) : 2 * (c0 + 2 * T)]
                        )
                        yrl_t[(pt, tau + 1)] = yrl
                u3 = u3_t.pop(tau)
                xre2, xim2 = [], []
                for pt in range(PTP):
                    yrl = yrl_t.pop((pt, tau))
                    yrc = p3.tile([128, T], f32, tag="yrc")
                    yic = p3.tile([128, T], f32, tag="yic")
                    V.scalar_tensor_tensor(
                        yrc[:], mptab_s(pt), GreA_t[1][:, pt, tau : tau + 1], yrl[:, 0:T],
                        op0=OP.mult, op1=OP.add,
                    )
                    V.scalar_tensor_tensor(
                        yic[:], mptab_s(pt), GimA_t[1][:, pt, tau : tau + 1], yrl[:, T : 2 * T],
                        op0=OP.mult, op1=OP.add,
                    )
                    u1 = p3.tile([128, T], f32, tag="u1")
                    u2 = p3.tile([128, T], f32, tag="u2")
                    u3r = p3.tile([128, T], f32, tag="u3r")
                    u4 = p3.tile([128, T], f32, tag="u4")
                    xr = p3.tile([128, T], bf16, tag="xr", bufs=10)
                    xi = p3.tile([128, T], bf16, tag="xi", bufs=10)
                    G.tensor_tensor(u1[:], ctab_s(pt), yrc[:], op=OP.mult)
                    G.tensor_tensor(u2[:], stab_s(pt), yic[:], op=OP.mult)
                    G.tensor_tensor(xr[:], u1[:], u2[:], op=OP.subtract)
                    G.tensor_tensor(u3r[:], stab_s(pt), yrc[:], op=OP.mult)
                    V.tensor_tensor(u4[:], ctab_s(pt), yic[:], op=OP.mult)
                    G.tensor_tensor(xi[:], u3r[:], u4[:], op=OP.add)
                    xre2.append(xr)
                    xim2.append(xi)
                ostage = p3.tile([128, HT * T], f32, tag="ostage", bufs=2)
                for hb in range(HT):
                    op_ = ps3.tile([128, T], f32, tag="o")
                    for k in range(PTP):
                        nc.tensor.matmul(
                            op_[:], ct[:, k * H + hb * 128 : k * H + (hb + 1) * 128], xre2[k][:],
                            start=(k == 0), stop=False,
                        )
                    for k in range(PTP):
                        nc.tensor.matmul(
                            op_[:], ct[:, (PTP + k) * H + hb * 128 : (PTP + k) * H + (hb + 1) * 128], xim2[k][:],
                            start=False, stop=(k == PTP - 1),
                        )
                    V.scalar_tensor_tensor(
                        ostage[:, hb * T : (hb + 1) * T],
                        u3[:, hb * T : (hb + 1) * T], dvec[:, hb : hb + 1], op_[:],
                        op0=OP.mult, op1=OP.add,
                    )
                nc.sync.dma_start(
                    io["outT"].ap()[:, :, c0 : c0 + T],
                    ostage[:].rearrange("p (h t) -> p h t", t=T),
                )

        _ctes.close()


def build_program(cfg):
    import concourse.bacc as bacc
    import concourse.mybir as mybir
    import concourse.tile as tile

    f32 = mybir.dt.float32
    f32r = mybir.dt.float32r
    bf16 = mybir.dt.bfloat16
    T = cfg["T"]
    S = cfg["L"] // NCORES
    KH = H // 128
    PTP = P // 128
    KR = R // 128
    HT = H // 128

    nc = bacc.Bacc(
        "TRN2", target_bir_lowering=False, debug=False, num_devices=NCORES
    )
    io = {}
    ins = [
        ("uT", (128, KH, S), bf16),
        ("BT", (128, 2 * KH * P), bf16),
        ("CT", (128, 2 * PTP * H), bf16),
        ("FT", (128, PTP * R), f32r),
        ("DW", (128, KR * R), f32r),
        ("ET", (128, KR * P), f32r),
        ("Dvec", (128, HT), f32),
        ("ctab", (128, PTP * T), bf16),
        ("stab", (128, PTP * T), bf16),
        ("mctab", (128, PTP * T), bf16),
        ("mstab", (128, PTP * T), bf16),
        ("mptab", (128, PTP * T), bf16),
        ("smalls", (128, PTP * SM_W), f32),
    ]
    for name, shape, dt_ in ins:
        io[name] = nc.dram_tensor(name, list(shape), dt_, kind="ExternalInput")
    io["outT"] = nc.dram_tensor("outT", [128, HT, S], f32, kind="ExternalOutput")

    with tile.TileContext(nc) as tc:
        _emit(nc, tc, io, cfg)
    nc.compile()
    return nc


# ======================================================================
# host side
# ======================================================================

def make_tables(lam_re, lam_im, cfg):
    T = cfg["T"]
    S = cfg["L"] // NCORES
    NSUB = S // T
    PTP = P // 128
    f32 = np.float32
    bfl = ml_dtypes.bfloat16
    lam = lam_re.astype(np.float64) + 1j * lam_im.astype(np.float64)
    mag = np.abs(lam)
    th = np.angle(lam)
    k = np.arange(T)
    ctab = np.cos(np.outer(th, k))          # (P, T) f64
    stab = np.sin(np.outer(th, k))
    mptab = mag[:, None] ** k[None, :]
    mctab = mptab * ctab
    mstab = mptab * stab
    tau = np.arange(NSUB)
    lpow = lam[:, None] ** (tau[None, :] * T + 1)

    def tile_pt(arr, dtype):
        # (P, W) -> (128, PTP*W): per-pt blocks of columns
        W = arr.shape[1]
        out = np.empty((128, PTP * W), dtype)
        for pt in range(PTP):
            out[:, pt * W : (pt + 1) * W] = arr[pt * 128 : (pt + 1) * 128]
        return out

    tabs = dict(
        ctab=tile_pt(ctab, bfl), stab=tile_pt(stab, bfl),
        mptab=tile_pt(mptab, bfl),
        mctab=tile_pt(mctab, bfl), mstab=tile_pt(mstab, bfl),
    )
    # smalls blob
    sm_full = np.zeros((P, SM_W), f32)
    sm_full[:, SM_MVEC] = mag
    sm_full[:, SM_COST] = np.cos(T * th)
    sm_full[:, SM_SINT] = np.sin(T * th)
    sm_full[:, SM_COST1] = np.cos((T - 1) * th)
    sm_full[:, SM_SINT1] = np.sin((T - 1) * th)
    sm_full[:, SM_LPR : SM_LPR + NSUB] = np.real(lpow)
    sm_full[:, SM_LPI : SM_LPI + NSUB] = np.imag(lpow)
    coefre = np.zeros((NCORES, P, NCORES), f32)
    coefim = np.zeros((NCORES, P, NCORES), f32)
    for m in range(NCORES):
        for j in range(m):
            v = lam ** (S * (m - 1 - j))
            coefre[m, :, j] = np.real(v)
            coefim[m, :, j] = np.imag(v)
    return tabs, sm_full, coefre, coefim


def make_in_maps(inputs, cfg):
    f32 = np.float32
    bfl = ml_dtypes.bfloat16
    Lc = cfg["L"]
    S = Lc // NCORES
    KH = H // 128
    PTP = P // 128
    KR = R // 128
    HT = H // 128
    u = np.ascontiguousarray(np.asarray(inputs["input_sequence"], f32)[:Lc])
    tabs, sm_full, coefre, coefim = make_tables(
        np.asarray(inputs["Lambda_re"]), np.asarray(inputs["Lambda_im"]), cfg
    )

    def pack_k(mat, nk, width, dtype):
        # mat: (nk*128, width) -> (128, nk*width)
        out = np.empty((128, nk * width), dtype)
        for k in range(nk):
            out[:, k * width : (k + 1) * width] = mat[k * 128 : (k + 1) * 128]
        return out

    BTre = np.asarray(inputs["B_re"], f32).T    # (H, P)
    BTim = np.asarray(inputs["B_im"], f32).T
    BT = np.concatenate(
        [pack_k(BTre, KH, P, bfl), pack_k(BTim, KH, P, bfl)], axis=1
    )
    CreT = np.asarray(inputs["C_re"], f32).T    # (P, H)
    nCimT = -np.asarray(inputs["C_im"], f32).T
    CT = np.concatenate(
        [pack_k(CreT, PTP, H, bfl), pack_k(nCimT, PTP, H, bfl)], axis=1
    )
    FT = pack_k(np.asarray(inputs["F"], f32).T, PTP, R, f32)       # (P,R) tiles
    DW = pack_k(np.asarray(inputs["Delta"], f32), KR, R, f32)      # (R,R) rows
    ET = pack_k(np.asarray(inputs["E"], f32).T, KR, P, f32)        # (R,P) tiles
    Dvec = pack_k(np.asarray(inputs["D"], f32)[:, None], HT, 1, f32)  # (128, HT)

    in_maps = []
    for m in range(NCORES):
        sm_m = sm_full.copy()
        sm_m[:, SM_ZCOL] = 0.0 if m == 0 else 1.0
        sm_m[:, SM_CRE : SM_CRE + NCORES] = coefre[m]
        sm_m[:, SM_CIM : SM_CIM + NCORES] = coefim[m]
        # tile smalls per pt
        sm_t = np.empty((128, PTP * SM_W), f32)
        for pt in range(PTP):
            sm_t[:, pt * SM_W : (pt + 1) * SM_W] = sm_m[pt * 128 : (pt + 1) * 128]
        uT = np.ascontiguousarray(u[m * S : (m + 1) * S, :].T)  # (H, S)
        uT_t = np.empty((128, KH, S), bfl)
        for k in range(KH):
            uT_t[:, k, :] = uT[k * 128 : (k + 1) * 128]
        im = dict(
            uT=uT_t, BT=BT, CT=CT, FT=FT, DW=DW, ET=ET, Dvec=Dvec,
            smalls=sm_t, **tabs,
        )
        in_maps.append(im)
    return in_maps


def assemble_output(results, cfg):
    Lc = cfg["L"]
    S = Lc // NCORES
    out = np.empty((Lc, H), np.float32)
    for m in range(NCORES):
        # outT: (128, HT, S) -> (S, H)
        o = results[m]["outT"]
        out[m * S : (m + 1) * S, :] = o.transpose(2, 1, 0).reshape(S, H)
    out[0, :] = 0.0
    return out


def get_program(cfg_key="full"):
    if cfg_key not in _PROG_CACHE:
        _PROG_CACHE[cfg_key] = build_program(CFG_FULL)
    return _PROG_CACHE[cfg_key]


def run(inputs, trace=False, **kw):
    from concourse import bass_utils

    nc = get_program()
    in_maps = make_in_maps(inputs, CFG_FULL)
    res = bass_utils.run_bass_kernel_spmd(
        nc, in_maps, core_ids=list(range(NCORES)), trace=trace, **kw
    )
    return assemble_output(res.results, CFG_FULL), res


def kernel(**inputs):
    out, _ = run(inputs)
    return out
